# revision 7
# baseline (speedup 1.0000x reference)
"""Causal self-attention (RoPE, GPT-J interleaved) Bass kernel for 8 TRN2 cores.

Sharding: core i handles batch b = i // 4 and heads [4*(i%4), 4*(i%4)+4).
Each core computes QKV for its head slice, attention, and a partial output
projection; the host sums the 4 partials per batch and adds b_proj.

Per-core dataflow (all SBUF tiles are [128 partitions, free]):
  phase 0: x [T, C] -> xT (8 tiles [128, T]) via PE transposes
  phase 1: qkvT[col, t] = w_qkv.T @ x.T via PE (weights stationary), bias
           folded into the PSUM->SBUF copy; RoPE applied on the q/k tiles
           (rotate = P @ q via PE matmul, then elementwise on DVE)
  phase 2: per head: scoresT[tk, tq] blocks -> exp on ACT -> PV accumulation
           into y [tq, 65] (col 64 = softmax denominator via ones column
           appended to v), per-partition reciprocal scaling
  phase 3: out[t, :] = yT.T @ w_o rows, DMA'd straight from PSUM
"""
import numpy as np
from contextlib import ExitStack

import concourse.bass as bass
import concourse.tile as tile
from concourse import bacc, mybir
from concourse import bass_utils
from concourse.masks import make_identity

F32 = mybir.dt.float32

N_CORES = 8
B = 2
T_FULL = 2048
C = 1024
H = 16
D = 64
HPC = 4          # heads per core
GROUPS = H // HPC  # 4 head-groups; core i -> batch i//GROUPS, group i%GROUPS
QKV_W = 3 * HPC * D  # 768 columns of the per-core qkv weight slice
MASK_VAL = -1e30


def _make_maskT(nc, mask_ap):
    # scoresT layout is [tk, tq]; valid (unmasked) when tq >= tk, i.e.
    # col >= row. Fill col < row with MASK_VAL.
    nc.gpsimd.memset(mask_ap, 0.0)
    nc.gpsimd.affine_select(
        out=mask_ap,
        in_=mask_ap,
        compare_op=mybir.AluOpType.is_ge,
        fill=MASK_VAL,
        base=0,
        # value = -row + col ; keep in_ where >= 0
        pattern=[[1, mask_ap.shape[1]]],
        channel_multiplier=-1,
    )


def emit_attention(ctx: ExitStack, tc: tile.TileContext, aps: dict, T: int,
                   mm_dt=F32):
    nc = tc.nc
    NT = T // 128       # number of 128-row tiles along T
    NCH = T // 512      # number of 512-wide chunks along T
    KT = C // 128       # 8 contraction tiles for QKV

    def mc(ap):
        return ap.bitcast(mm_dt) if mm_dt != F32 else ap

    x_d, wq_d, bqT_d, wo_d, cos2_d, sin2_d, pmatT_d, out_d = (
        aps["x"], aps["w_qkv"], aps["b_qkvT"], aps["w_o"],
        aps["cos2"], aps["sin2"], aps["pmatT"], aps["out"])

    const = ctx.enter_context(tc.tile_pool(name="const", bufs=1))
    qk_pool = ctx.enter_context(tc.tile_pool(name="qk", bufs=1))
    vaug_pool = ctx.enter_context(tc.tile_pool(name="vaug", bufs=1))
    y_pool = ctx.enter_context(tc.tile_pool(name="ystage", bufs=1))

    ident = const.tile([128, 128], F32, tag="ident")
    make_identity(nc, ident)
    maskT = const.tile([128, 128], F32, tag="maskT")
    _make_maskT(nc, maskT)

    wo = []
    for p in range(2):
        w = const.tile([128, C], F32, tag=f"wo{p}")
        nc.sync.dma_start(w, wo_d[p * 128:(p + 1) * 128, :])
        wo.append(w)

    # long-lived activation tensors
    qkT = [qk_pool.tile([128, T], F32, tag=f"qkT{i}", name=f"qkT{i}")
           for i in range(4)]
    vaug = [vaug_pool.tile([128, NT, 65], F32, tag=f"vaug{h}",
                           name=f"vaug{h}") for h in range(HPC)]
    for h in range(HPC):
        nc.vector.memset(vaug[h][:, :, 64:65], 1.0)

    # -------- phases 0+1 (scoped: xT/wq/cos/sin freed before phase 2) ----
    with tc.tile_pool(name="ph01", bufs=1) as ph01, \
         tc.tile_pool(name="ph0ld", bufs=3) as xload, \
         tc.tile_pool(name="ph01ps", bufs=4, space="PSUM") as tp_ps, \
         tc.tile_pool(name="ph1ps", bufs=3, space="PSUM") as qkv_ps:
        pmatT = ph01.tile([128, 128], F32, tag="pmatT")
        nc.sync.dma_start(pmatT, pmatT_d)
        cos2 = ph01.tile([128, T], F32, tag="cos2")
        nc.sync.dma_start(cos2, cos2_d)
        sin2 = ph01.tile([128, T], F32, tag="sin2")
        nc.sync.dma_start(sin2, sin2_d)
        bqT = ph01.tile([128, 6], F32, tag="bqT")
        nc.sync.dma_start(bqT, bqT_d)
        wq = []
        for k in range(KT):
            w = ph01.tile([128, QKV_W], F32, tag=f"wq{k}", name=f"wq{k}")
            nc.sync.dma_start(w, wq_d[k * 128:(k + 1) * 128, :])
            wq.append(w)

        # phase 0: build xT (8 tiles [128, T]) via PE transposes
        xT = [ph01.tile([128, T], F32, tag=f"xT{k}", name=f"xT{k}")
              for k in range(KT)]
        for r in range(NT):
            xr = xload.tile([128, C], F32, tag="xr")
            nc.sync.dma_start(xr, x_d[r * 128:(r + 1) * 128, :])
            for k in range(KT):
                pt = tp_ps.tile([128, 512], F32, tag="tp", name="pt")
                nc.tensor.transpose(pt[:, 0:128], xr[:, k * 128:(k + 1) * 128], ident)
                nc.vector.tensor_copy(xT[k][:, r * 128:(r + 1) * 128], pt[:, 0:128])

        # phase 1: qkvT + bias + RoPE; v transposed into v_aug
        # col-tile layout of wq columns: [q01 | q23 | k01 | k23 | v01 | v23]
        vT = [ph01.tile([128, T], F32, tag=f"vT{i}", name=f"vT{i}")
              for i in range(2)]
        for ct in range(6):
            dest = qkT[ct] if ct < 4 else vT[ct - 4]
            for ch in range(NCH):
                sl = slice(ch * 512, (ch + 1) * 512)
                ps = qkv_ps.tile([128, 512], F32, tag="qkv")
                for k in range(KT):
                    nc.tensor.matmul(
                        ps, mc(wq[k][:, ct * 128:(ct + 1) * 128]),
                        mc(xT[k][:, sl]),
                        start=(k == 0), stop=(k == KT - 1))
                # PSUM -> SBUF copy with per-partition bias add
                nc.vector.tensor_scalar_add(dest[:, sl], ps,
                                            bqT[:, ct:ct + 1])
                if ct < 4:
                    # RoPE on this chunk: rot = P @ q, then
                    # q = q*cos + rot*sin (all elementwise on DVE)
                    rp = tp_ps.tile([128, 512], F32, tag="tp", name="rp")
                    nc.tensor.matmul(rp, mc(pmatT), mc(dest[:, sl]),
                                     start=True, stop=True)
                    nc.vector.tensor_tensor(rp, rp, sin2[:, sl],
                                            op=mybir.AluOpType.mult)
                    nc.vector.tensor_tensor(dest[:, sl], dest[:, sl],
                                            cos2[:, sl],
                                            op=mybir.AluOpType.mult)
                    nc.vector.tensor_tensor(dest[:, sl], dest[:, sl], rp,
                                            op=mybir.AluOpType.add)
        # v: transpose vT pair blocks into per-head v_aug tiles
        for p in range(2):
            for r in range(NT):
                pt = tp_ps.tile([128, 512], F32, tag="tp", name="pt")
                nc.tensor.transpose(pt[:, 0:128], vT[p][:, r * 128:(r + 1) * 128],
                                    ident)
                nc.vector.tensor_copy(vaug[2 * p][:, r, 0:64], pt[:, 0:64])
                nc.vector.tensor_copy(vaug[2 * p + 1][:, r, 0:64],
                                      pt[:, 64:128])

    # -------- phase 2: per-head attention --------
    ypair = [y_pool.tile([128, T], F32, tag=f"ypair{p}", name=f"ypair{p}") for p in range(2)]
    with tc.tile_pool(name="exps", bufs=1) as exp_pool, \
         tc.tile_pool(name="recips", bufs=4) as recip_pool, \
         tc.tile_pool(name="scps", bufs=3, space="PSUM") as sc_ps, \
         tc.tile_pool(name="yps", bufs=3, space="PSUM") as y_ps_pool:
        for h in range(HPC):
            hp, hl = h // 2, h % 2
            rows = slice(hl * 64, (hl + 1) * 64)
            kT_t, qT_t = qkT[2 + hp], qkT[hp]
            # stage 1: scoresT_j = k_j . q  -> mask -> exp (expT_j in SBUF)
            expT = []
            for j in range(NT):
                g0 = 128 * j
                nj = T - g0
                e = exp_pool.tile([128, nj], F32, tag=f"exp{j}", name=f"exp{j}")
                expT.append(e)
                c0 = g0
                while c0 < T:
                    c1 = min(T, (c0 // 512 + 1) * 512)
                    w = c1 - c0
                    ps = sc_ps.tile([128, w], F32, tag="sc")
                    nc.tensor.matmul(
                        ps, mc(kT_t[rows, g0:g0 + 128]),
                        mc(qT_t[rows, c0:c1]), start=True, stop=True)
                    if c0 == g0:
                        nc.vector.tensor_tensor(ps[:, 0:128], ps[:, 0:128],
                                                maskT,
                                                op=mybir.AluOpType.add)
                    nc.scalar.activation(e[:, c0 - g0:c1 - g0], ps,
                                         mybir.ActivationFunctionType.Exp,
                                         scale=0.125)
                    c0 = c1
            # stage 2: PV accumulation per query tile + denominator scale
            for r in range(NT):
                yp = y_ps_pool.tile([128, 65], F32, tag="y")
                for j in range(r + 1):
                    off = (r - j) * 128
                    nc.tensor.matmul(yp, mc(expT[j][:, off:off + 128]),
                                     mc(vaug[h][:, j, :]),
                                     start=(j == 0), stop=(j == r))
                rc = recip_pool.tile([128, 1], F32, tag="rc")
                nc.vector.reciprocal(rc, yp[:, 64:65])
                nc.vector.tensor_scalar_mul(
                    ypair[hp][:, r * 128 + hl * 64: r * 128 + hl * 64 + 64],
                    yp[:, 0:64], rc)

    # -------- phase 2b/3: y -> yT, out = yT.T @ w_o --------
    with tc.tile_pool(name="yT", bufs=1) as yT_pool, \
         tc.tile_pool(name="outsb", bufs=3) as out_sb_pool, \
         tc.tile_pool(name="ph3tp", bufs=2, space="PSUM") as tp_ps3, \
         tc.tile_pool(name="outps", bufs=4, space="PSUM") as out_ps:
        yT = [yT_pool.tile([128, T], F32, tag=f"yT{p}", name=f"yT{p}") for p in range(2)]
        for p in range(2):
            for r in range(NT):
                pt = tp_ps3.tile([128, 128], F32, tag="ytp")
                nc.tensor.transpose(pt, ypair[p][:, r * 128:(r + 1) * 128],
                                    ident)
                nc.vector.tensor_copy(yT[p][:, r * 128:(r + 1) * 128], pt)
        for r in range(NT):
            for nchk in range(C // 512):
                po = out_ps.tile([128, 512], F32, tag="po")
                for p in range(2):
                    nc.tensor.matmul(
                        po, mc(yT[p][:, r * 128:(r + 1) * 128]),
                        mc(wo[p][:, nchk * 512:(nchk + 1) * 512]),
                        start=(p == 0), stop=(p == 1))
                osb = out_sb_pool.tile([128, 512], F32, tag="osb")
                nc.vector.tensor_copy(osb, po)
                nc.sync.dma_start(
                    out_d[r * 128:(r + 1) * 128,
                          nchk * 512:(nchk + 1) * 512], osb)


def build_nc(T=T_FULL, mm_dt=F32):
    nc = bacc.Bacc("TRN2", target_bir_lowering=False, debug=False,
                   enable_asserts=False, num_devices=N_CORES)
    aps = {}
    aps["x"] = nc.dram_tensor("x", [T, C], F32, kind="ExternalInput").ap()
    aps["w_qkv"] = nc.dram_tensor("w_qkv", [C, QKV_W], F32,
                                  kind="ExternalInput").ap()
    aps["b_qkvT"] = nc.dram_tensor("b_qkvT", [128, 6], F32,
                                   kind="ExternalInput").ap()
    aps["w_o"] = nc.dram_tensor("w_o", [HPC * D, C], F32,
                                kind="ExternalInput").ap()
    aps["cos2"] = nc.dram_tensor("cos2", [128, T], F32,
                                 kind="ExternalInput").ap()
    aps["sin2"] = nc.dram_tensor("sin2", [128, T], F32,
                                 kind="ExternalInput").ap()
    aps["pmatT"] = nc.dram_tensor("pmatT", [128, 128], F32,
                                  kind="ExternalInput").ap()
    aps["out"] = nc.dram_tensor("out", [T, C], F32,
                                kind="ExternalOutput").ap()
    with tile.TileContext(nc) as tc:
        with ExitStack() as ctx:
            emit_attention(ctx, tc, aps, T, mm_dt)
    nc.compile()
    return nc


def rope_tables(T=T_FULL):
    """cos/sin tables exactly as reference.py builds them (f32 arithmetic),
    stacked for the 2-head [128, T] tile layout."""
    try:
        import jax
        import jax.numpy as jnp
        with jax.default_device(jax.devices("cpu")[0]):
            inv_freq = 1.0 / (10000.0 ** (
                jnp.arange(0, D, 2, dtype=jnp.float32) / D))
            t = jnp.arange(T, dtype=jnp.float32)
            freqs = t[:, None] * inv_freq[None, :]
            emb = jnp.concatenate((freqs, freqs), axis=-1)
            cos = np.asarray(jnp.cos(emb), dtype=np.float32)
            sin = np.asarray(jnp.sin(emb), dtype=np.float32)
    except Exception:
        inv_freq = (1.0 / (10000.0 ** (
            np.arange(0, D, 2, dtype=np.float64) / D))).astype(np.float32)
        t = np.arange(T, dtype=np.float32)
        freqs = (t[:, None] * inv_freq[None, :]).astype(np.float32)
        emb = np.concatenate((freqs, freqs), axis=-1)
        cos = np.cos(emb, dtype=np.float32)
        sin = np.sin(emb, dtype=np.float32)
    cos2 = np.vstack([cos.T, cos.T]).astype(np.float32)   # [128, T]
    sin2 = np.vstack([sin.T, sin.T]).astype(np.float32)
    return np.ascontiguousarray(cos2), np.ascontiguousarray(sin2)


def pmat_T():
    # rot(q) = P @ q along the head dim: P[2i, 2i+1] = -1, P[2i+1, 2i] = 1,
    # block-diagonal over the two stacked heads. Pass P.T as matmul lhsT.
    P = np.zeros((64, 64), np.float32)
    for i in range(32):
        P[2 * i, 2 * i + 1] = -1.0
        P[2 * i + 1, 2 * i] = 1.0
    P128 = np.zeros((128, 128), np.float32)
    P128[0:64, 0:64] = P
    P128[64:128, 64:128] = P
    return np.ascontiguousarray(P128.T)


def shard_inputs(x, w_attn, b_attn, w_proj, T=T_FULL):
    """Build the 8 per-core input maps."""
    cos2, sin2 = rope_tables(T)
    pT = pmat_T()
    in_maps = []
    for core in range(N_CORES):
        b = core // GROUPS
        g = core % GROUPS
        h0 = g * HPC
        cols = slice(h0 * D, (h0 + HPC) * D)
        w_qkv = np.concatenate(
            [w_attn[:, cols], w_attn[:, C:][:, cols],
             w_attn[:, 2 * C:][:, cols]], axis=1)
        b_qkv = np.concatenate(
            [b_attn[cols], b_attn[C:][cols], b_attn[2 * C:][cols]])
        b_qkvT = np.ascontiguousarray(
            b_qkv.reshape(6, 128).T)            # [128, 6], col-tile major
        w_o = w_proj[cols, :]
        in_maps.append({
            "x": np.ascontiguousarray(x[b], dtype=np.float32),
            "w_qkv": np.ascontiguousarray(w_qkv, dtype=np.float32),
            "b_qkvT": np.ascontiguousarray(b_qkvT, dtype=np.float32),
            "w_o": np.ascontiguousarray(w_o, dtype=np.float32),
            "cos2": cos2,
            "sin2": sin2,
            "pmatT": pT,
        })
    return in_maps


_NC_CACHE = {}


def kernel(x, w_attn, b_attn, w_proj, b_proj):
    x = np.asarray(x, dtype=np.float32)
    w_attn = np.asarray(w_attn, dtype=np.float32)
    b_attn = np.asarray(b_attn, dtype=np.float32)
    w_proj = np.asarray(w_proj, dtype=np.float32)
    b_proj = np.asarray(b_proj, dtype=np.float32)

    key = ("main", T_FULL)
    if key not in _NC_CACHE:
        _NC_CACHE[key] = build_nc(T_FULL)
    nc = _NC_CACHE[key]

    in_maps = shard_inputs(x, w_attn, b_attn, w_proj, T_FULL)
    res = bass_utils.run_bass_kernel_spmd(
        nc, in_maps, core_ids=list(range(N_CORES)))
    out = np.zeros((B, T_FULL, C), dtype=np.float32)
    for core in range(N_CORES):
        out[core // GROUPS] += res.results[core]["out"]
    out += b_proj[None, None, :]
    return out


# revision 10
# speedup vs baseline: 1.1962x; 1.1962x over previous
"""Causal self-attention (RoPE, GPT-J interleaved) Bass kernel for 8 TRN2 cores.

Sharding: core i handles batch b = i // 4 and heads [4*(i%4), 4*(i%4)+4).
Each core computes QKV for its head slice, attention, and a partial output
projection; the host sums the 4 partials per batch and adds b_proj.

Per-core dataflow (all SBUF tiles are [128 partitions, free]):
  phase 0: x [T, C] -> xT (8 tiles [128, T]) via PE transposes
  phase 1: qkvT[col, t] = w_qkv.T @ x.T via PE (weights stationary), bias
           folded into the PSUM->SBUF copy; RoPE applied on the q/k tiles
           (rotate = P @ q via PE matmul, then elementwise on DVE)
  phase 2: per head: scoresT[tk, tq] blocks -> exp on ACT -> PV accumulation
           into y [tq, 65] (col 64 = softmax denominator via ones column
           appended to v), per-partition reciprocal scaling
  phase 3: out[t, :] = yT.T @ w_o rows, DMA'd straight from PSUM
"""
import numpy as np
from contextlib import ExitStack

import concourse.bass as bass
import concourse.tile as tile
from concourse import bacc, mybir
from concourse import bass_utils
from concourse.masks import make_identity

F32 = mybir.dt.float32

N_CORES = 8
B = 2
T_FULL = 2048
C = 1024
H = 16
D = 64
HPC = 4          # heads per core
GROUPS = H // HPC  # 4 head-groups; core i -> batch i//GROUPS, group i%GROUPS
QKV_W = 3 * HPC * D  # 768 columns of the per-core qkv weight slice
MASK_VAL = -1e30


def _make_maskT(nc, mask_ap):
    # scoresT layout is [tk, tq]; valid (unmasked) when tq >= tk, i.e.
    # col >= row. Fill col < row with MASK_VAL.
    nc.gpsimd.memset(mask_ap, 0.0)
    nc.gpsimd.affine_select(
        out=mask_ap,
        in_=mask_ap,
        compare_op=mybir.AluOpType.is_ge,
        fill=MASK_VAL,
        base=0,
        # value = -row + col ; keep in_ where >= 0
        pattern=[[1, mask_ap.shape[1]]],
        channel_multiplier=-1,
    )


def emit_attention(ctx: ExitStack, tc: tile.TileContext, aps: dict, T: int,
                   mm_dt=F32):
    nc = tc.nc
    NT = T // 128       # number of 128-row tiles along T
    NCH = T // 512      # number of 512-wide chunks along T
    KT = C // 128       # 8 contraction tiles for QKV

    def mc(ap):
        return ap.bitcast(mm_dt) if mm_dt != F32 else ap

    x_d, wq_d, bqT_d, wo_d, cos2_d, sin2_d, pmatT_d, out_d = (
        aps["x"], aps["w_qkv"], aps["b_qkvT"], aps["w_o"],
        aps["cos2"], aps["sin2"], aps["pmatT"], aps["out"])

    const = ctx.enter_context(tc.tile_pool(name="const", bufs=1))
    qk_pool = ctx.enter_context(tc.tile_pool(name="qk", bufs=1))
    vaug_pool = ctx.enter_context(tc.tile_pool(name="vaug", bufs=1))
    y_pool = ctx.enter_context(tc.tile_pool(name="ystage", bufs=1))

    ident = const.tile([128, 128], F32, tag="ident")
    make_identity(nc, ident)
    maskT = const.tile([128, 128], F32, tag="maskT")
    _make_maskT(nc, maskT)

    wo = []
    for p in range(2):
        w = const.tile([128, C], F32, tag=f"wo{p}")
        nc.sync.dma_start(w, wo_d[p * 128:(p + 1) * 128, :])
        wo.append(w)

    # long-lived activation tensors
    qkT = [qk_pool.tile([128, T], F32, tag=f"qkT{i}", name=f"qkT{i}")
           for i in range(4)]
    vaug = [vaug_pool.tile([128, NT, 65], F32, tag=f"vaug{h}",
                           name=f"vaug{h}") for h in range(HPC)]
    for h in range(HPC):
        nc.vector.memset(vaug[h][:, :, 64:65], 1.0)

    # -------- phases 0+1 (scoped: xT/wq/cos/sin freed before phase 2) ----
    with tc.tile_pool(name="ph01", bufs=1) as ph01, \
         tc.tile_pool(name="ph0ld", bufs=3) as xload, \
         tc.tile_pool(name="ph01ps", bufs=4, space="PSUM") as tp_ps, \
         tc.tile_pool(name="ph1ps", bufs=3, space="PSUM") as qkv_ps:
        pmatT = ph01.tile([128, 128], F32, tag="pmatT")
        nc.sync.dma_start(pmatT, pmatT_d)
        cos2 = ph01.tile([128, T], F32, tag="cos2")
        nc.sync.dma_start(cos2, cos2_d)
        sin2 = ph01.tile([128, T], F32, tag="sin2")
        nc.sync.dma_start(sin2, sin2_d)
        bqT = ph01.tile([128, 6], F32, tag="bqT")
        nc.sync.dma_start(bqT, bqT_d)
        wq = []
        for k in range(KT):
            w = ph01.tile([128, QKV_W], F32, tag=f"wq{k}", name=f"wq{k}")
            nc.sync.dma_start(w, wq_d[k * 128:(k + 1) * 128, :])
            wq.append(w)

        # phase 0: build xT (8 tiles [128, T]) via PE transposes
        xT = [ph01.tile([128, T], F32, tag=f"xT{k}", name=f"xT{k}")
              for k in range(KT)]
        for r in range(NT):
            xr = xload.tile([128, C], F32, tag="xr")
            nc.sync.dma_start(xr, x_d[r * 128:(r + 1) * 128, :])
            for k in range(KT):
                pt = tp_ps.tile([128, 512], F32, tag="tp", name="pt")
                nc.tensor.transpose(pt[:, 0:128], xr[:, k * 128:(k + 1) * 128], ident)
                nc.vector.tensor_copy(xT[k][:, r * 128:(r + 1) * 128], pt[:, 0:128])

        # phase 1: qkvT + bias + RoPE; v transposed into v_aug
        # col-tile layout of wq columns: [q01 | q23 | k01 | k23 | v01 | v23]
        vT = [ph01.tile([128, T], F32, tag=f"vT{i}", name=f"vT{i}")
              for i in range(2)]
        for ct in range(6):
            dest = qkT[ct] if ct < 4 else vT[ct - 4]
            for ch in range(NCH):
                sl = slice(ch * 512, (ch + 1) * 512)
                ps = qkv_ps.tile([128, 512], F32, tag="qkv")
                for k in range(KT):
                    nc.tensor.matmul(
                        ps, mc(wq[k][:, ct * 128:(ct + 1) * 128]),
                        mc(xT[k][:, sl]),
                        start=(k == 0), stop=(k == KT - 1))
                # PSUM -> SBUF copy with per-partition bias add
                nc.vector.tensor_scalar_add(dest[:, sl], ps,
                                            bqT[:, ct:ct + 1])
                if ct < 4:
                    # RoPE on this chunk: rot = P @ q, then
                    # q = q*cos + rot*sin (all elementwise on DVE)
                    rp = tp_ps.tile([128, 512], F32, tag="tp", name="rp")
                    nc.tensor.matmul(rp, mc(pmatT), mc(dest[:, sl]),
                                     start=True, stop=True)
                    nc.vector.tensor_tensor(rp, rp, sin2[:, sl],
                                            op=mybir.AluOpType.mult)
                    nc.vector.tensor_tensor(dest[:, sl], dest[:, sl],
                                            cos2[:, sl],
                                            op=mybir.AluOpType.mult)
                    nc.vector.tensor_tensor(dest[:, sl], dest[:, sl], rp,
                                            op=mybir.AluOpType.add)
        # v: transpose vT pair blocks into per-head v_aug tiles
        for p in range(2):
            for r in range(NT):
                pt = tp_ps.tile([128, 512], F32, tag="tp", name="pt")
                nc.tensor.transpose(pt[:, 0:128], vT[p][:, r * 128:(r + 1) * 128],
                                    ident)
                nc.vector.tensor_copy(vaug[2 * p][:, r, 0:64], pt[:, 0:64])
                nc.vector.tensor_copy(vaug[2 * p + 1][:, r, 0:64],
                                      pt[:, 64:128])

    # -------- phase 2: per-head attention --------
    ypair = [y_pool.tile([128, T], F32, tag=f"ypair{p}", name=f"ypair{p}") for p in range(2)]
    with tc.tile_pool(name="exps", bufs=1) as exp_pool, \
         tc.tile_pool(name="recips", bufs=4) as recip_pool, \
         tc.tile_pool(name="scps", bufs=3, space="PSUM") as sc_ps, \
         tc.tile_pool(name="yps", bufs=3, space="PSUM") as y_ps_pool:
        for h in range(HPC):
            hp, hl = h // 2, h % 2
            rows = slice(hl * 64, (hl + 1) * 64)
            kT_t, qT_t = qkT[2 + hp], qkT[hp]
            # stage 1: scoresT_j = k_j . q  -> mask -> exp (expT_j in SBUF)
            expT = []
            for j in range(NT):
                g0 = 128 * j
                nj = T - g0
                e = exp_pool.tile([128, nj], F32, tag=f"exp{j}", name=f"exp{j}")
                expT.append(e)
                c0 = g0
                while c0 < T:
                    c1 = min(T, (c0 // 512 + 1) * 512)
                    w = c1 - c0
                    ps = sc_ps.tile([128, w], F32, tag="sc")
                    nc.tensor.matmul(
                        ps, mc(kT_t[rows, g0:g0 + 128]),
                        mc(qT_t[rows, c0:c1]), start=True, stop=True)
                    if c0 == g0:
                        nc.vector.tensor_tensor(ps[:, 0:128], ps[:, 0:128],
                                                maskT,
                                                op=mybir.AluOpType.add)
                    nc.scalar.activation(e[:, c0 - g0:c1 - g0], ps,
                                         mybir.ActivationFunctionType.Exp,
                                         scale=0.125)
                    c0 = c1
            # stage 2: PV accumulation per query tile + denominator scale
            for r in range(NT):
                yp = y_ps_pool.tile([128, 65], F32, tag="y")
                for j in range(r + 1):
                    off = (r - j) * 128
                    nc.tensor.matmul(yp, mc(expT[j][:, off:off + 128]),
                                     mc(vaug[h][:, j, :]),
                                     start=(j == 0), stop=(j == r))
                rc = recip_pool.tile([128, 1], F32, tag="rc")
                nc.vector.reciprocal(rc, yp[:, 64:65])
                nc.vector.tensor_scalar_mul(
                    ypair[hp][:, r * 128 + hl * 64: r * 128 + hl * 64 + 64],
                    yp[:, 0:64], rc)

    # -------- phase 2b/3: y -> yT, out = yT.T @ w_o --------
    with tc.tile_pool(name="yT", bufs=1) as yT_pool, \
         tc.tile_pool(name="outsb", bufs=3) as out_sb_pool, \
         tc.tile_pool(name="ph3tp", bufs=2, space="PSUM") as tp_ps3, \
         tc.tile_pool(name="outps", bufs=4, space="PSUM") as out_ps:
        yT = [yT_pool.tile([128, T], F32, tag=f"yT{p}", name=f"yT{p}") for p in range(2)]
        for p in range(2):
            for r in range(NT):
                pt = tp_ps3.tile([128, 128], F32, tag="ytp")
                nc.tensor.transpose(pt, ypair[p][:, r * 128:(r + 1) * 128],
                                    ident)
                nc.vector.tensor_copy(yT[p][:, r * 128:(r + 1) * 128], pt)
        for r in range(NT):
            for nchk in range(C // 512):
                po = out_ps.tile([128, 512], F32, tag="po")
                for p in range(2):
                    nc.tensor.matmul(
                        po, mc(yT[p][:, r * 128:(r + 1) * 128]),
                        mc(wo[p][:, nchk * 512:(nchk + 1) * 512]),
                        start=(p == 0), stop=(p == 1))
                osb = out_sb_pool.tile([128, 512], F32, tag="osb")
                nc.vector.tensor_copy(osb, po)
                nc.sync.dma_start(
                    out_d[r * 128:(r + 1) * 128,
                          nchk * 512:(nchk + 1) * 512], osb)


def build_nc(T=T_FULL, mm_dt=F32):
    nc = bacc.Bacc("TRN2", target_bir_lowering=False, debug=False,
                   enable_asserts=False, num_devices=N_CORES)
    aps = {}
    aps["x"] = nc.dram_tensor("x", [T, C], F32, kind="ExternalInput").ap()
    aps["w_qkv"] = nc.dram_tensor("w_qkv", [C, QKV_W], F32,
                                  kind="ExternalInput").ap()
    aps["b_qkvT"] = nc.dram_tensor("b_qkvT", [128, 6], F32,
                                   kind="ExternalInput").ap()
    aps["w_o"] = nc.dram_tensor("w_o", [HPC * D, C], F32,
                                kind="ExternalInput").ap()
    aps["cos2"] = nc.dram_tensor("cos2", [128, T], F32,
                                 kind="ExternalInput").ap()
    aps["sin2"] = nc.dram_tensor("sin2", [128, T], F32,
                                 kind="ExternalInput").ap()
    aps["pmatT"] = nc.dram_tensor("pmatT", [128, 128], F32,
                                  kind="ExternalInput").ap()
    aps["out"] = nc.dram_tensor("out", [T, C], F32,
                                kind="ExternalOutput").ap()
    with tile.TileContext(nc) as tc:
        with ExitStack() as ctx:
            emit_attention(ctx, tc, aps, T, mm_dt)
    nc.compile()
    return nc


def rope_tables(T=T_FULL):
    """cos/sin tables exactly as reference.py builds them (f32 arithmetic),
    stacked for the 2-head [128, T] tile layout."""
    try:
        import jax
        import jax.numpy as jnp
        with jax.default_device(jax.devices("cpu")[0]):
            inv_freq = 1.0 / (10000.0 ** (
                jnp.arange(0, D, 2, dtype=jnp.float32) / D))
            t = jnp.arange(T, dtype=jnp.float32)
            freqs = t[:, None] * inv_freq[None, :]
            emb = jnp.concatenate((freqs, freqs), axis=-1)
            cos = np.asarray(jnp.cos(emb), dtype=np.float32)
            sin = np.asarray(jnp.sin(emb), dtype=np.float32)
    except Exception:
        inv_freq = (1.0 / (10000.0 ** (
            np.arange(0, D, 2, dtype=np.float64) / D))).astype(np.float32)
        t = np.arange(T, dtype=np.float32)
        freqs = (t[:, None] * inv_freq[None, :]).astype(np.float32)
        emb = np.concatenate((freqs, freqs), axis=-1)
        cos = np.cos(emb, dtype=np.float32)
        sin = np.sin(emb, dtype=np.float32)
    cos2 = np.vstack([cos.T, cos.T]).astype(np.float32)   # [128, T]
    sin2 = np.vstack([sin.T, sin.T]).astype(np.float32)
    return np.ascontiguousarray(cos2), np.ascontiguousarray(sin2)


def pmat_T():
    # rot(q) = P @ q along the head dim: P[2i, 2i+1] = -1, P[2i+1, 2i] = 1,
    # block-diagonal over the two stacked heads. Pass P.T as matmul lhsT.
    P = np.zeros((64, 64), np.float32)
    for i in range(32):
        P[2 * i, 2 * i + 1] = -1.0
        P[2 * i + 1, 2 * i] = 1.0
    P128 = np.zeros((128, 128), np.float32)
    P128[0:64, 0:64] = P
    P128[64:128, 64:128] = P
    return np.ascontiguousarray(P128.T)


def shard_inputs(x, w_attn, b_attn, w_proj, T=T_FULL):
    """Build the 8 per-core input maps."""
    cos2, sin2 = rope_tables(T)
    pT = pmat_T()
    in_maps = []
    for core in range(N_CORES):
        b = core // GROUPS
        g = core % GROUPS
        h0 = g * HPC
        cols = slice(h0 * D, (h0 + HPC) * D)
        w_qkv = np.concatenate(
            [w_attn[:, cols], w_attn[:, C:][:, cols],
             w_attn[:, 2 * C:][:, cols]], axis=1)
        b_qkv = np.concatenate(
            [b_attn[cols], b_attn[C:][cols], b_attn[2 * C:][cols]])
        b_qkvT = np.ascontiguousarray(
            b_qkv.reshape(6, 128).T)            # [128, 6], col-tile major
        w_o = w_proj[cols, :]
        in_maps.append({
            "x": np.ascontiguousarray(x[b], dtype=np.float32),
            "w_qkv": np.ascontiguousarray(w_qkv, dtype=np.float32),
            "b_qkvT": np.ascontiguousarray(b_qkvT, dtype=np.float32),
            "w_o": np.ascontiguousarray(w_o, dtype=np.float32),
            "cos2": cos2,
            "sin2": sin2,
            "pmatT": pT,
        })
    return in_maps


_NC_CACHE = {}


def kernel(x, w_attn, b_attn, w_proj, b_proj):
    x = np.asarray(x, dtype=np.float32)
    w_attn = np.asarray(w_attn, dtype=np.float32)
    b_attn = np.asarray(b_attn, dtype=np.float32)
    w_proj = np.asarray(w_proj, dtype=np.float32)
    b_proj = np.asarray(b_proj, dtype=np.float32)

    key = ("main", T_FULL)
    if key not in _NC_CACHE:
        _NC_CACHE[key] = build_nc(T_FULL)
    nc = _NC_CACHE[key]

    in_maps = shard_inputs(x, w_attn, b_attn, w_proj, T_FULL)
    res = bass_utils.run_bass_kernel_spmd(
        nc, in_maps, core_ids=list(range(N_CORES)))
    out = np.zeros((B, T_FULL, C), dtype=np.float32)
    for core in range(N_CORES):
        out[core // GROUPS] += res.results[core]["out"]
    out += b_proj[None, None, :]
    return out


F16 = mybir.dt.float16
F32R = mybir.dt.float32r


def emit_attention_v2(ctx: ExitStack, tc: tile.TileContext, aps: dict, T: int,
                      prec: str = "f32"):
    """v2: PV keeps v_aug stationary and accumulates yT [65, T] directly
    (softmax denominator in row 64); per-column scale via a PE ones-row
    broadcast; no y transposes. prec: f32 | f32r | f16 selects the dtype of
    all matmul operand tiles (psum accumulation is always f32)."""
    nc = tc.nc
    NT = T // 128
    NCH = T // 512
    KT = C // 128
    f16 = prec == "f16"
    DT = {"f32": F32, "f32r": F32R, "f16": F16}[prec]

    x_d, wq_d, bqT_d, wo_d, cos2_d, sin2_d, pmatT_d, out_d = (
        aps["x"], aps["w_qkv"], aps["b_qkvT"], aps["w_o"],
        aps["cos2"], aps["sin2"], aps["pmatT"], aps["out"])

    const = ctx.enter_context(tc.tile_pool(name="const", bufs=1))
    qk_pool = ctx.enter_context(tc.tile_pool(name="qk", bufs=1))
    vaug_pool = ctx.enter_context(tc.tile_pool(name="vaug", bufs=1))
    y_pool = ctx.enter_context(tc.tile_pool(name="ystage", bufs=1))

    ident = const.tile([128, 128], F32, tag="ident")
    make_identity(nc, ident)
    maskT = const.tile([128, 128], F32, tag="maskT")
    _make_maskT(nc, maskT)
    ones_row = const.tile([1, 64], F32, tag="ones_row")
    nc.vector.memset(ones_row, 1.0)

    wo = []
    for p in range(2):
        w = const.tile([128, C], DT, tag=f"wo{p}", name=f"wo{p}")
        nc.sync.dma_start(w, wo_d[p * 128:(p + 1) * 128, :])
        wo.append(w)

    # matmul-operand versions of the RoPE'd q/k tiles
    qkT16 = [qk_pool.tile([128, T], DT, tag=f"qkT16_{i}", name=f"qkT16_{i}")
             for i in range(4)]
    vaug = [vaug_pool.tile([128, NT, 65], DT, tag=f"vaug{h}",
                           name=f"vaug{h}") for h in range(HPC)]
    for h in range(HPC):
        nc.vector.memset(vaug[h][:, :, 64:65], 1.0)
    # ysb: scaled yT pair tiles feeding the out-projection
    ysb = [y_pool.tile([128, T], DT, tag=f"ysb{p}", name=f"ysb{p}")
           for p in range(2)]

    # -------- phases 0+1 --------
    with tc.tile_pool(name="ph01", bufs=1) as ph01, \
         tc.tile_pool(name="ph0ld", bufs=3) as xload, \
         tc.tile_pool(name="ph01ps", bufs=4, space="PSUM") as tp_ps, \
         tc.tile_pool(name="ph1ps", bufs=3, space="PSUM") as qkv_ps:
        pmatT = ph01.tile([128, 128], DT, tag="pmatT")
        nc.sync.dma_start(pmatT, pmatT_d)
        cos2 = ph01.tile([128, T], F32, tag="cos2")
        nc.sync.dma_start(cos2, cos2_d)
        sin2 = ph01.tile([128, T], F32, tag="sin2")
        nc.sync.dma_start(sin2, sin2_d)
        bqT = ph01.tile([128, 6], F32, tag="bqT")
        nc.sync.dma_start(bqT, bqT_d)
        wq = []
        for k in range(KT):
            w = ph01.tile([128, QKV_W], DT, tag=f"wq{k}", name=f"wq{k}")
            nc.sync.dma_start(w, wq_d[k * 128:(k + 1) * 128, :])
            wq.append(w)

        # phase 0: xT tiles [128, T] in DT
        xT = [ph01.tile([128, T], DT, tag=f"xT{k}", name=f"xT{k}")
              for k in range(KT)]
        if f16:
            # x arrives f16 in DRAM; DMA-transpose straight into SBUF
            for k in range(KT):
                nc.sync.dma_start_transpose(
                    xT[k], x_d[:, k * 128:(k + 1) * 128])
        else:
            for r in range(NT):
                xr = xload.tile([128, C], F32, tag="xr")
                nc.sync.dma_start(xr, x_d[r * 128:(r + 1) * 128, :])
                for k in range(KT):
                    pt = tp_ps.tile([128, 512], F32, tag="tp", name="pt")
                    nc.tensor.transpose(pt[:, 0:128],
                                        xr[:, k * 128:(k + 1) * 128], ident)
                    nc.vector.tensor_copy(xT[k][:, r * 128:(r + 1) * 128],
                                          pt[:, 0:128])

        # phase 1: qkvT chunks; q/k RoPE in f32 then cast into qkT16
        cast_qk = DT != F32
        qkT = [ph01.tile([128, T], F32, tag=f"qkTf{i}", name=f"qkTf{i}")
               for i in range(4)] if cast_qk else qkT16
        vT = [ph01.tile([128, T], F32, tag=f"vT{i}", name=f"vT{i}")
              for i in range(2)]
        for ct in range(6):
            dest = qkT[ct] if ct < 4 else vT[ct - 4]
            for ch in range(NCH):
                sl = slice(ch * 512, (ch + 1) * 512)
                ps = qkv_ps.tile([128, 512], F32, tag="qkv")
                for k in range(KT):
                    nc.tensor.matmul(
                        ps, wq[k][:, ct * 128:(ct + 1) * 128], xT[k][:, sl],
                        start=(k == 0), stop=(k == KT - 1))
                nc.vector.tensor_scalar_add(dest[:, sl], ps,
                                            bqT[:, ct:ct + 1])
                if ct < 4:
                    # RoPE: rot = P @ q (PE), q = q*cos + rot*sin (DVE)
                    rope_src = ph01.tile([128, 512], DT, tag="ropesrc",
                                         name="ropesrc") if cast_qk else None
                    if cast_qk:
                        nc.vector.tensor_copy(rope_src, dest[:, sl])
                        rhs_ap = rope_src
                    else:
                        rhs_ap = dest[:, sl]
                    rp = tp_ps.tile([128, 512], F32, tag="tp", name="rp")
                    nc.tensor.matmul(rp, pmatT, rhs_ap,
                                     start=True, stop=True)
                    nc.vector.tensor_tensor(rp, rp, sin2[:, sl],
                                            op=mybir.AluOpType.mult)
                    nc.vector.tensor_tensor(dest[:, sl], dest[:, sl],
                                            cos2[:, sl],
                                            op=mybir.AluOpType.mult)
                    if cast_qk:
                        nc.vector.tensor_tensor(qkT16[ct][:, sl],
                                                dest[:, sl], rp,
                                                op=mybir.AluOpType.add)
                    else:
                        nc.vector.tensor_tensor(dest[:, sl], dest[:, sl],
                                                rp, op=mybir.AluOpType.add)
        # v: transpose vT pair blocks into per-head v_aug tiles (cast to DT)
        for p in range(2):
            for r in range(NT):
                pt = tp_ps.tile([128, 512], F32, tag="tp", name="pt")
                nc.tensor.transpose(pt[:, 0:128],
                                    vT[p][:, r * 128:(r + 1) * 128], ident)
                nc.vector.tensor_copy(vaug[2 * p][:, r, 0:64], pt[:, 0:64])
                nc.vector.tensor_copy(vaug[2 * p + 1][:, r, 0:64],
                                      pt[:, 64:128])

    # -------- phase 2: per-head attention --------
    with tc.tile_pool(name="exps", bufs=1) as exp_pool, \
         tc.tile_pool(name="fin", bufs=2) as fin_pool, \
         tc.tile_pool(name="scps", bufs=3, space="PSUM") as sc_ps, \
         tc.tile_pool(name="bcps", bufs=1, space="PSUM") as bc_ps, \
         tc.tile_pool(name="ytps", bufs=1, space="PSUM") as yt_ps_pool:
        for h in range(HPC):
            hp, hl = h // 2, h % 2
            rows = slice(hl * 64, (hl + 1) * 64)
            kT_t, qT_t = qkT16[2 + hp], qkT16[hp]
            expT = []
            for j in range(NT):
                g0 = 128 * j
                nj = T - g0
                e = exp_pool.tile([128, nj], DT, tag=f"exp{j}",
                                  name=f"exp{j}")
                expT.append(e)
                c0 = g0
                while c0 < T:
                    c1 = min(T, (c0 // 512 + 1) * 512)
                    w = c1 - c0
                    ps = sc_ps.tile([128, w], F32, tag="sc", name="sc")
                    nc.tensor.matmul(
                        ps, kT_t[rows, g0:g0 + 128], qT_t[rows, c0:c1],
                        start=True, stop=True)
                    if c0 == g0:
                        nc.vector.tensor_tensor(ps[:, 0:128], ps[:, 0:128],
                                                maskT,
                                                op=mybir.AluOpType.add)
                    nc.scalar.activation(e[:, c0 - g0:c1 - g0], ps,
                                         mybir.ActivationFunctionType.Exp,
                                         scale=0.125)
                    c0 = c1
            # PV: yT accumulation with v_aug stationary
            yt_ps = yt_ps_pool.tile([65, T], F32, tag="ytps", name="ytps")
            for j in range(NT):
                g0 = 128 * j
                c0 = g0
                while c0 < T:
                    c1 = min(T, (c0 // 512 + 1) * 512)
                    bank = c0 // 512
                    j_last = min(4 * bank + 3, NT - 1)
                    nc.tensor.matmul(
                        yt_ps[:, c0:c1], vaug[h][:, j, :],
                        expT[j][:, c0 - g0:c1 - g0],
                        start=(j == 0), stop=(j == j_last))
                    c0 = c1
            # finalize: per-column scale by 1/denominator (row 64)
            densb = fin_pool.tile([1, T], F32, tag="densb", name="densb")
            nc.vector.tensor_copy(densb, yt_ps[64:65, :])
            recr = fin_pool.tile([1, T], F32, tag="recr", name="recr")
            nc.vector.reciprocal(recr, densb)
            for ch in range(NCH):
                sl = slice(ch * 512, (ch + 1) * 512)
                bc = bc_ps.tile([64, 512], F32, tag="bc", name="bc")
                nc.tensor.matmul(bc, ones_row, recr[:, sl],
                                 start=True, stop=True)
                nc.vector.tensor_tensor(ysb[hp][rows, sl], yt_ps[0:64, sl],
                                        bc, op=mybir.AluOpType.mult)

    # -------- phase 3: out = ysb.T @ w_o --------
    with tc.tile_pool(name="outsb", bufs=3) as out_sb_pool, \
         tc.tile_pool(name="outps", bufs=4, space="PSUM") as out_ps:
        for r in range(NT):
            for nchk in range(C // 512):
                po = out_ps.tile([128, 512], F32, tag="po", name="po")
                for p in range(2):
                    nc.tensor.matmul(
                        po, ysb[p][:, r * 128:(r + 1) * 128],
                        wo[p][:, nchk * 512:(nchk + 1) * 512],
                        start=(p == 0), stop=(p == 1))
                osb = out_sb_pool.tile([128, 512], F32, tag="osb",
                                       name="osb")
                nc.scalar.copy(osb, po)
                nc.sync.dma_start(
                    out_d[r * 128:(r + 1) * 128,
                          nchk * 512:(nchk + 1) * 512], osb)


def build_nc_v2(T=T_FULL, prec="f32"):
    f16 = prec == "f16"
    DTD = {"f32": F32, "f32r": F32R, "f16": F16}[prec]
    nc = bacc.Bacc("TRN2", target_bir_lowering=False, debug=False,
                   enable_asserts=False, num_devices=N_CORES)
    aps = {}
    # f32r path loads x as f32 (PE transpose + rounding DVE copy produce f32r)
    x_dt = F32 if prec == "f32r" else DTD
    aps["x"] = nc.dram_tensor("x", [T, C], x_dt, kind="ExternalInput").ap()
    aps["w_qkv"] = nc.dram_tensor("w_qkv", [C, QKV_W], DTD,
                                  kind="ExternalInput").ap()
    aps["b_qkvT"] = nc.dram_tensor("b_qkvT", [128, 6], F32,
                                   kind="ExternalInput").ap()
    aps["w_o"] = nc.dram_tensor("w_o", [HPC * D, C], DTD,
                                kind="ExternalInput").ap()
    aps["cos2"] = nc.dram_tensor("cos2", [128, T], F32,
                                 kind="ExternalInput").ap()
    aps["sin2"] = nc.dram_tensor("sin2", [128, T], F32,
                                 kind="ExternalInput").ap()
    aps["pmatT"] = nc.dram_tensor("pmatT", [128, 128], DTD,
                                  kind="ExternalInput").ap()
    aps["out"] = nc.dram_tensor("out", [T, C], F32,
                                kind="ExternalOutput").ap()
    with tile.TileContext(nc) as tc:
        with ExitStack() as ctx:
            emit_attention_v2(ctx, tc, aps, T, prec)
    nc.compile()
    return nc


def shard_inputs_v2(x, w_attn, b_attn, w_proj, T=T_FULL, prec="f32"):
    in_maps = shard_inputs(x, w_attn, b_attn, w_proj, T)
    np_dt = np.float16 if prec == "f16" else np.float32
    for m in in_maps:
        for key in ("x", "w_qkv", "w_o", "pmatT"):
            m[key] = np.ascontiguousarray(m[key].astype(np_dt))
    return in_maps


# revision 12
# speedup vs baseline: 1.2856x; 1.0748x over previous
"""Causal self-attention (RoPE, GPT-J interleaved) Bass kernel for 8 TRN2 cores.

Sharding: core i handles batch b = i // 4 and heads [4*(i%4), 4*(i%4)+4).
Each core computes QKV for its head slice, attention, and a partial output
projection; the host sums the 4 partials per batch and adds b_proj.

Per-core dataflow (all SBUF tiles are [128 partitions, free]):
  phase 0: x [T, C] -> xT (8 tiles [128, T]) via PE transposes
  phase 1: qkvT[col, t] = w_qkv.T @ x.T via PE (weights stationary), bias
           folded into the PSUM->SBUF copy; RoPE applied on the q/k tiles
           (rotate = P @ q via PE matmul, then elementwise on DVE)
  phase 2: per head: scoresT[tk, tq] blocks -> exp on ACT -> PV accumulation
           into y [tq, 65] (col 64 = softmax denominator via ones column
           appended to v), per-partition reciprocal scaling
  phase 3: out[t, :] = yT.T @ w_o rows, DMA'd straight from PSUM
"""
import numpy as np
from contextlib import ExitStack

import concourse.bass as bass
import concourse.tile as tile
from concourse import bacc, mybir
from concourse import bass_utils
from concourse.masks import make_identity

F32 = mybir.dt.float32

N_CORES = 8
B = 2
T_FULL = 2048
C = 1024
H = 16
D = 64
HPC = 4          # heads per core
GROUPS = H // HPC  # 4 head-groups; core i -> batch i//GROUPS, group i%GROUPS
QKV_W = 3 * HPC * D  # 768 columns of the per-core qkv weight slice
MASK_VAL = -1e30


def _make_maskT(nc, mask_ap):
    # scoresT layout is [tk, tq]; valid (unmasked) when tq >= tk, i.e.
    # col >= row. Fill col < row with MASK_VAL.
    nc.gpsimd.memset(mask_ap, 0.0)
    nc.gpsimd.affine_select(
        out=mask_ap,
        in_=mask_ap,
        compare_op=mybir.AluOpType.is_ge,
        fill=MASK_VAL,
        base=0,
        # value = -row + col ; keep in_ where >= 0
        pattern=[[1, mask_ap.shape[1]]],
        channel_multiplier=-1,
    )


def emit_attention(ctx: ExitStack, tc: tile.TileContext, aps: dict, T: int,
                   mm_dt=F32):
    nc = tc.nc
    NT = T // 128       # number of 128-row tiles along T
    NCH = T // 512      # number of 512-wide chunks along T
    KT = C // 128       # 8 contraction tiles for QKV

    def mc(ap):
        return ap.bitcast(mm_dt) if mm_dt != F32 else ap

    x_d, wq_d, bqT_d, wo_d, cos2_d, sin2_d, pmatT_d, out_d = (
        aps["x"], aps["w_qkv"], aps["b_qkvT"], aps["w_o"],
        aps["cos2"], aps["sin2"], aps["pmatT"], aps["out"])

    const = ctx.enter_context(tc.tile_pool(name="const", bufs=1))
    qk_pool = ctx.enter_context(tc.tile_pool(name="qk", bufs=1))
    vaug_pool = ctx.enter_context(tc.tile_pool(name="vaug", bufs=1))
    y_pool = ctx.enter_context(tc.tile_pool(name="ystage", bufs=1))

    ident = const.tile([128, 128], F32, tag="ident")
    make_identity(nc, ident)
    maskT = const.tile([128, 128], F32, tag="maskT")
    _make_maskT(nc, maskT)

    wo = []
    for p in range(2):
        w = const.tile([128, C], F32, tag=f"wo{p}")
        nc.sync.dma_start(w, wo_d[p * 128:(p + 1) * 128, :])
        wo.append(w)

    # long-lived activation tensors
    qkT = [qk_pool.tile([128, T], F32, tag=f"qkT{i}", name=f"qkT{i}")
           for i in range(4)]
    vaug = [vaug_pool.tile([128, NT, 65], F32, tag=f"vaug{h}",
                           name=f"vaug{h}") for h in range(HPC)]
    for h in range(HPC):
        nc.vector.memset(vaug[h][:, :, 64:65], 1.0)

    # -------- phases 0+1 (scoped: xT/wq/cos/sin freed before phase 2) ----
    with tc.tile_pool(name="ph01", bufs=1) as ph01, \
         tc.tile_pool(name="ph0ld", bufs=3) as xload, \
         tc.tile_pool(name="ph01ps", bufs=4, space="PSUM") as tp_ps, \
         tc.tile_pool(name="ph1ps", bufs=3, space="PSUM") as qkv_ps:
        pmatT = ph01.tile([128, 128], F32, tag="pmatT")
        nc.sync.dma_start(pmatT, pmatT_d)
        cos2 = ph01.tile([128, T], F32, tag="cos2")
        nc.sync.dma_start(cos2, cos2_d)
        sin2 = ph01.tile([128, T], F32, tag="sin2")
        nc.sync.dma_start(sin2, sin2_d)
        bqT = ph01.tile([128, 6], F32, tag="bqT")
        nc.sync.dma_start(bqT, bqT_d)
        wq = []
        for k in range(KT):
            w = ph01.tile([128, QKV_W], F32, tag=f"wq{k}", name=f"wq{k}")
            nc.sync.dma_start(w, wq_d[k * 128:(k + 1) * 128, :])
            wq.append(w)

        # phase 0: build xT (8 tiles [128, T]) via PE transposes
        xT = [ph01.tile([128, T], F32, tag=f"xT{k}", name=f"xT{k}")
              for k in range(KT)]
        for r in range(NT):
            xr = xload.tile([128, C], F32, tag="xr")
            nc.sync.dma_start(xr, x_d[r * 128:(r + 1) * 128, :])
            for k in range(KT):
                pt = tp_ps.tile([128, 512], F32, tag="tp", name="pt")
                nc.tensor.transpose(pt[:, 0:128], xr[:, k * 128:(k + 1) * 128], ident)
                nc.vector.tensor_copy(xT[k][:, r * 128:(r + 1) * 128], pt[:, 0:128])

        # phase 1: qkvT + bias + RoPE; v transposed into v_aug
        # col-tile layout of wq columns: [q01 | q23 | k01 | k23 | v01 | v23]
        vT = [ph01.tile([128, T], F32, tag=f"vT{i}", name=f"vT{i}")
              for i in range(2)]
        for ct in range(6):
            dest = qkT[ct] if ct < 4 else vT[ct - 4]
            for ch in range(NCH):
                sl = slice(ch * 512, (ch + 1) * 512)
                ps = qkv_ps.tile([128, 512], F32, tag="qkv")
                for k in range(KT):
                    nc.tensor.matmul(
                        ps, mc(wq[k][:, ct * 128:(ct + 1) * 128]),
                        mc(xT[k][:, sl]),
                        start=(k == 0), stop=(k == KT - 1))
                # PSUM -> SBUF copy with per-partition bias add
                nc.vector.tensor_scalar_add(dest[:, sl], ps,
                                            bqT[:, ct:ct + 1])
                if ct < 4:
                    # RoPE on this chunk: rot = P @ q, then
                    # q = q*cos + rot*sin (all elementwise on DVE)
                    rp = tp_ps.tile([128, 512], F32, tag="tp", name="rp")
                    nc.tensor.matmul(rp, mc(pmatT), mc(dest[:, sl]),
                                     start=True, stop=True)
                    nc.vector.tensor_tensor(rp, rp, sin2[:, sl],
                                            op=mybir.AluOpType.mult)
                    nc.vector.tensor_tensor(dest[:, sl], dest[:, sl],
                                            cos2[:, sl],
                                            op=mybir.AluOpType.mult)
                    nc.vector.tensor_tensor(dest[:, sl], dest[:, sl], rp,
                                            op=mybir.AluOpType.add)
        # v: transpose vT pair blocks into per-head v_aug tiles
        for p in range(2):
            for r in range(NT):
                pt = tp_ps.tile([128, 512], F32, tag="tp", name="pt")
                nc.tensor.transpose(pt[:, 0:128], vT[p][:, r * 128:(r + 1) * 128],
                                    ident)
                nc.vector.tensor_copy(vaug[2 * p][:, r, 0:64], pt[:, 0:64])
                nc.vector.tensor_copy(vaug[2 * p + 1][:, r, 0:64],
                                      pt[:, 64:128])

    # -------- phase 2: per-head attention --------
    ypair = [y_pool.tile([128, T], F32, tag=f"ypair{p}", name=f"ypair{p}") for p in range(2)]
    with tc.tile_pool(name="exps", bufs=1) as exp_pool, \
         tc.tile_pool(name="recips", bufs=4) as recip_pool, \
         tc.tile_pool(name="scps", bufs=3, space="PSUM") as sc_ps, \
         tc.tile_pool(name="yps", bufs=3, space="PSUM") as y_ps_pool:
        for h in range(HPC):
            hp, hl = h // 2, h % 2
            rows = slice(hl * 64, (hl + 1) * 64)
            kT_t, qT_t = qkT[2 + hp], qkT[hp]
            # stage 1: scoresT_j = k_j . q  -> mask -> exp (expT_j in SBUF)
            expT = []
            for j in range(NT):
                g0 = 128 * j
                nj = T - g0
                e = exp_pool.tile([128, nj], F32, tag=f"exp{j}", name=f"exp{j}")
                expT.append(e)
                c0 = g0
                while c0 < T:
                    c1 = min(T, (c0 // 512 + 1) * 512)
                    w = c1 - c0
                    ps = sc_ps.tile([128, w], F32, tag="sc")
                    nc.tensor.matmul(
                        ps, mc(kT_t[rows, g0:g0 + 128]),
                        mc(qT_t[rows, c0:c1]), start=True, stop=True)
                    if c0 == g0:
                        nc.vector.tensor_tensor(ps[:, 0:128], ps[:, 0:128],
                                                maskT,
                                                op=mybir.AluOpType.add)
                    nc.scalar.activation(e[:, c0 - g0:c1 - g0], ps,
                                         mybir.ActivationFunctionType.Exp,
                                         scale=0.125)
                    c0 = c1
            # stage 2: PV accumulation per query tile + denominator scale
            for r in range(NT):
                yp = y_ps_pool.tile([128, 65], F32, tag="y")
                for j in range(r + 1):
                    off = (r - j) * 128
                    nc.tensor.matmul(yp, mc(expT[j][:, off:off + 128]),
                                     mc(vaug[h][:, j, :]),
                                     start=(j == 0), stop=(j == r))
                rc = recip_pool.tile([128, 1], F32, tag="rc")
                nc.vector.reciprocal(rc, yp[:, 64:65])
                nc.vector.tensor_scalar_mul(
                    ypair[hp][:, r * 128 + hl * 64: r * 128 + hl * 64 + 64],
                    yp[:, 0:64], rc)

    # -------- phase 2b/3: y -> yT, out = yT.T @ w_o --------
    with tc.tile_pool(name="yT", bufs=1) as yT_pool, \
         tc.tile_pool(name="outsb", bufs=3) as out_sb_pool, \
         tc.tile_pool(name="ph3tp", bufs=2, space="PSUM") as tp_ps3, \
         tc.tile_pool(name="outps", bufs=4, space="PSUM") as out_ps:
        yT = [yT_pool.tile([128, T], F32, tag=f"yT{p}", name=f"yT{p}") for p in range(2)]
        for p in range(2):
            for r in range(NT):
                pt = tp_ps3.tile([128, 128], F32, tag="ytp")
                nc.tensor.transpose(pt, ypair[p][:, r * 128:(r + 1) * 128],
                                    ident)
                nc.vector.tensor_copy(yT[p][:, r * 128:(r + 1) * 128], pt)
        for r in range(NT):
            for nchk in range(C // 512):
                po = out_ps.tile([128, 512], F32, tag="po")
                for p in range(2):
                    nc.tensor.matmul(
                        po, mc(yT[p][:, r * 128:(r + 1) * 128]),
                        mc(wo[p][:, nchk * 512:(nchk + 1) * 512]),
                        start=(p == 0), stop=(p == 1))
                osb = out_sb_pool.tile([128, 512], F32, tag="osb")
                nc.vector.tensor_copy(osb, po)
                nc.sync.dma_start(
                    out_d[r * 128:(r + 1) * 128,
                          nchk * 512:(nchk + 1) * 512], osb)


def build_nc(T=T_FULL, mm_dt=F32):
    nc = bacc.Bacc("TRN2", target_bir_lowering=False, debug=False,
                   enable_asserts=False, num_devices=N_CORES)
    aps = {}
    aps["x"] = nc.dram_tensor("x", [T, C], F32, kind="ExternalInput").ap()
    aps["w_qkv"] = nc.dram_tensor("w_qkv", [C, QKV_W], F32,
                                  kind="ExternalInput").ap()
    aps["b_qkvT"] = nc.dram_tensor("b_qkvT", [128, 6], F32,
                                   kind="ExternalInput").ap()
    aps["w_o"] = nc.dram_tensor("w_o", [HPC * D, C], F32,
                                kind="ExternalInput").ap()
    aps["cos2"] = nc.dram_tensor("cos2", [128, T], F32,
                                 kind="ExternalInput").ap()
    aps["sin2"] = nc.dram_tensor("sin2", [128, T], F32,
                                 kind="ExternalInput").ap()
    aps["pmatT"] = nc.dram_tensor("pmatT", [128, 128], F32,
                                  kind="ExternalInput").ap()
    aps["out"] = nc.dram_tensor("out", [T, C], F32,
                                kind="ExternalOutput").ap()
    with tile.TileContext(nc) as tc:
        with ExitStack() as ctx:
            emit_attention(ctx, tc, aps, T, mm_dt)
    nc.compile()
    return nc


def rope_tables(T=T_FULL):
    """cos/sin tables exactly as reference.py builds them (f32 arithmetic),
    stacked for the 2-head [128, T] tile layout."""
    try:
        import jax
        import jax.numpy as jnp
        with jax.default_device(jax.devices("cpu")[0]):
            inv_freq = 1.0 / (10000.0 ** (
                jnp.arange(0, D, 2, dtype=jnp.float32) / D))
            t = jnp.arange(T, dtype=jnp.float32)
            freqs = t[:, None] * inv_freq[None, :]
            emb = jnp.concatenate((freqs, freqs), axis=-1)
            cos = np.asarray(jnp.cos(emb), dtype=np.float32)
            sin = np.asarray(jnp.sin(emb), dtype=np.float32)
    except Exception:
        inv_freq = (1.0 / (10000.0 ** (
            np.arange(0, D, 2, dtype=np.float64) / D))).astype(np.float32)
        t = np.arange(T, dtype=np.float32)
        freqs = (t[:, None] * inv_freq[None, :]).astype(np.float32)
        emb = np.concatenate((freqs, freqs), axis=-1)
        cos = np.cos(emb, dtype=np.float32)
        sin = np.sin(emb, dtype=np.float32)
    cos2 = np.vstack([cos.T, cos.T]).astype(np.float32)   # [128, T]
    sin2 = np.vstack([sin.T, sin.T]).astype(np.float32)
    return np.ascontiguousarray(cos2), np.ascontiguousarray(sin2)


def pmat_T():
    # rot(q) = P @ q along the head dim: P[2i, 2i+1] = -1, P[2i+1, 2i] = 1,
    # block-diagonal over the two stacked heads. Pass P.T as matmul lhsT.
    P = np.zeros((64, 64), np.float32)
    for i in range(32):
        P[2 * i, 2 * i + 1] = -1.0
        P[2 * i + 1, 2 * i] = 1.0
    P128 = np.zeros((128, 128), np.float32)
    P128[0:64, 0:64] = P
    P128[64:128, 64:128] = P
    return np.ascontiguousarray(P128.T)


def shard_inputs(x, w_attn, b_attn, w_proj, T=T_FULL):
    """Build the 8 per-core input maps."""
    cos2, sin2 = rope_tables(T)
    pT = pmat_T()
    in_maps = []
    for core in range(N_CORES):
        b = core // GROUPS
        g = core % GROUPS
        h0 = g * HPC
        cols = slice(h0 * D, (h0 + HPC) * D)
        w_qkv = np.concatenate(
            [w_attn[:, cols], w_attn[:, C:][:, cols],
             w_attn[:, 2 * C:][:, cols]], axis=1)
        b_qkv = np.concatenate(
            [b_attn[cols], b_attn[C:][cols], b_attn[2 * C:][cols]])
        b_qkvT = np.ascontiguousarray(
            b_qkv.reshape(6, 128).T)            # [128, 6], col-tile major
        w_o = w_proj[cols, :]
        in_maps.append({
            "x": np.ascontiguousarray(x[b], dtype=np.float32),
            "w_qkv": np.ascontiguousarray(w_qkv, dtype=np.float32),
            "b_qkvT": np.ascontiguousarray(b_qkvT, dtype=np.float32),
            "w_o": np.ascontiguousarray(w_o, dtype=np.float32),
            "cos2": cos2,
            "sin2": sin2,
            "pmatT": pT,
        })
    return in_maps


_NC_CACHE = {}

# Selected variant for the graded kernel() entry point:
#   ("v1", None) = fp32 baseline, ("v2", "f32"|"f32r"|"f16") = restructured
KERNEL_CONFIG = ("v1", None)


def kernel(x, w_attn, b_attn, w_proj, b_proj):
    x = np.asarray(x, dtype=np.float32)
    w_attn = np.asarray(w_attn, dtype=np.float32)
    b_attn = np.asarray(b_attn, dtype=np.float32)
    w_proj = np.asarray(w_proj, dtype=np.float32)
    b_proj = np.asarray(b_proj, dtype=np.float32)

    version, prec = KERNEL_CONFIG
    key = (version, prec, T_FULL)
    if key not in _NC_CACHE:
        _NC_CACHE[key] = (build_nc(T_FULL) if version == "v1"
                          else build_nc_v2(T_FULL, prec))
    nc = _NC_CACHE[key]

    if version == "v1":
        in_maps = shard_inputs(x, w_attn, b_attn, w_proj, T_FULL)
    else:
        in_maps = shard_inputs_v2(x, w_attn, b_attn, w_proj, T_FULL, prec)
    res = bass_utils.run_bass_kernel_spmd(
        nc, in_maps, core_ids=list(range(N_CORES)))
    out = np.zeros((B, T_FULL, C), dtype=np.float32)
    for core in range(N_CORES):
        out[core // GROUPS] += res.results[core]["out"]
    out += b_proj[None, None, :]
    return out


F16 = mybir.dt.float16
F32R = mybir.dt.float32r


def emit_attention_v2(ctx: ExitStack, tc: tile.TileContext, aps: dict, T: int,
                      prec: str = "f32"):
    """v2: PV keeps v_aug stationary and accumulates yT [65, T] directly
    (softmax denominator in row 64); per-column scale via a PE ones-row
    broadcast; no y transposes. prec: f32 | f32r | f16 selects the dtype of
    all matmul operand tiles (psum accumulation is always f32)."""
    nc = tc.nc
    NT = T // 128
    NCH = T // 512
    KT = C // 128
    f16 = prec == "f16"
    DT = {"f32": F32, "f32r": F32R, "f16": F16}[prec]

    x_d, wq_d, bqT_d, wo_d, cos2_d, sin2_d, pmatT_d, out_d = (
        aps["x"], aps["w_qkv"], aps["b_qkvT"], aps["w_o"],
        aps["cos2"], aps["sin2"], aps["pmatT"], aps["out"])

    const = ctx.enter_context(tc.tile_pool(name="const", bufs=1))
    qk_pool = ctx.enter_context(tc.tile_pool(name="qk", bufs=1))
    vaug_pool = ctx.enter_context(tc.tile_pool(name="vaug", bufs=1))
    y_pool = ctx.enter_context(tc.tile_pool(name="ystage", bufs=1))

    ident = const.tile([128, 128], F32, tag="ident")
    make_identity(nc, ident)
    maskT = const.tile([128, 128], F32, tag="maskT")
    _make_maskT(nc, maskT)
    ones_row = const.tile([1, 64], F32, tag="ones_row")
    nc.vector.memset(ones_row, 1.0)

    wo = []
    for p in range(2):
        w = const.tile([128, C], DT, tag=f"wo{p}", name=f"wo{p}")
        nc.sync.dma_start(w, wo_d[p * 128:(p + 1) * 128, :])
        wo.append(w)

    # matmul-operand versions of the RoPE'd q/k tiles
    qkT16 = [qk_pool.tile([128, T], DT, tag=f"qkT16_{i}", name=f"qkT16_{i}")
             for i in range(4)]
    vaug = [vaug_pool.tile([128, NT, 65], DT, tag=f"vaug{h}",
                           name=f"vaug{h}") for h in range(HPC)]
    for h in range(HPC):
        nc.vector.memset(vaug[h][:, :, 64:65], 1.0)
    # ysb: scaled yT pair tiles feeding the out-projection
    ysb = [y_pool.tile([128, T], DT, tag=f"ysb{p}", name=f"ysb{p}")
           for p in range(2)]

    # -------- phases 0+1 --------
    with tc.tile_pool(name="ph01", bufs=1) as ph01, \
         tc.tile_pool(name="ph0ld", bufs=3) as xload, \
         tc.tile_pool(name="ph01ps", bufs=4, space="PSUM") as tp_ps, \
         tc.tile_pool(name="ph1ps", bufs=3, space="PSUM") as qkv_ps:
        pmatT = ph01.tile([128, 128], DT, tag="pmatT")
        nc.sync.dma_start(pmatT, pmatT_d)
        cos2 = ph01.tile([128, T], F32, tag="cos2")
        nc.sync.dma_start(cos2, cos2_d)
        sin2 = ph01.tile([128, T], F32, tag="sin2")
        nc.sync.dma_start(sin2, sin2_d)
        bqT = ph01.tile([128, 6], F32, tag="bqT")
        nc.sync.dma_start(bqT, bqT_d)
        wq = []
        for k in range(KT):
            w = ph01.tile([128, QKV_W], DT, tag=f"wq{k}", name=f"wq{k}")
            nc.sync.dma_start(w, wq_d[k * 128:(k + 1) * 128, :])
            wq.append(w)

        # phase 0: xT tiles [128, T] in DT
        xT = [ph01.tile([128, T], DT, tag=f"xT{k}", name=f"xT{k}")
              for k in range(KT)]
        if f16:
            # x arrives f16 in DRAM; DMA-transpose straight into SBUF
            for k in range(KT):
                nc.sync.dma_start_transpose(
                    xT[k], x_d[:, k * 128:(k + 1) * 128])
        else:
            for r in range(NT):
                xr = xload.tile([128, C], F32, tag="xr")
                nc.sync.dma_start(xr, x_d[r * 128:(r + 1) * 128, :])
                for k in range(KT):
                    pt = tp_ps.tile([128, 512], F32, tag="tp", name="pt")
                    nc.tensor.transpose(pt[:, 0:128],
                                        xr[:, k * 128:(k + 1) * 128], ident)
                    nc.vector.tensor_copy(xT[k][:, r * 128:(r + 1) * 128],
                                          pt[:, 0:128])

        # phase 1: qkvT chunks; q/k RoPE in f32 then cast into qkT16
        cast_qk = DT != F32
        qkT = [ph01.tile([128, T], F32, tag=f"qkTf{i}", name=f"qkTf{i}")
               for i in range(4)] if cast_qk else qkT16
        vT = [ph01.tile([128, T], F32, tag=f"vT{i}", name=f"vT{i}")
              for i in range(2)]
        for ct in range(6):
            dest = qkT[ct] if ct < 4 else vT[ct - 4]
            for ch in range(NCH):
                sl = slice(ch * 512, (ch + 1) * 512)
                ps = qkv_ps.tile([128, 512], F32, tag="qkv")
                for k in range(KT):
                    nc.tensor.matmul(
                        ps, wq[k][:, ct * 128:(ct + 1) * 128], xT[k][:, sl],
                        start=(k == 0), stop=(k == KT - 1))
                nc.vector.tensor_scalar_add(dest[:, sl], ps,
                                            bqT[:, ct:ct + 1])
                if ct < 4:
                    # RoPE: rot = P @ q (PE), q = q*cos + rot*sin (DVE)
                    rope_src = ph01.tile([128, 512], DT, tag="ropesrc",
                                         name="ropesrc") if cast_qk else None
                    if cast_qk:
                        nc.vector.tensor_copy(rope_src, dest[:, sl])
                        rhs_ap = rope_src
                    else:
                        rhs_ap = dest[:, sl]
                    rp = tp_ps.tile([128, 512], F32, tag="tp", name="rp")
                    nc.tensor.matmul(rp, pmatT, rhs_ap,
                                     start=True, stop=True)
                    nc.vector.tensor_tensor(rp, rp, sin2[:, sl],
                                            op=mybir.AluOpType.mult)
                    nc.vector.tensor_tensor(dest[:, sl], dest[:, sl],
                                            cos2[:, sl],
                                            op=mybir.AluOpType.mult)
                    if cast_qk:
                        nc.vector.tensor_tensor(qkT16[ct][:, sl],
                                                dest[:, sl], rp,
                                                op=mybir.AluOpType.add)
                    else:
                        nc.vector.tensor_tensor(dest[:, sl], dest[:, sl],
                                                rp, op=mybir.AluOpType.add)
        # v: transpose vT pair blocks into per-head v_aug tiles (cast to DT)
        for p in range(2):
            for r in range(NT):
                pt = tp_ps.tile([128, 512], F32, tag="tp", name="pt")
                nc.tensor.transpose(pt[:, 0:128],
                                    vT[p][:, r * 128:(r + 1) * 128], ident)
                nc.vector.tensor_copy(vaug[2 * p][:, r, 0:64], pt[:, 0:64])
                nc.vector.tensor_copy(vaug[2 * p + 1][:, r, 0:64],
                                      pt[:, 64:128])

    # -------- phase 2: per-head attention --------
    with tc.tile_pool(name="exps", bufs=1) as exp_pool, \
         tc.tile_pool(name="fin", bufs=2) as fin_pool, \
         tc.tile_pool(name="scps", bufs=3, space="PSUM") as sc_ps, \
         tc.tile_pool(name="bcps", bufs=1, space="PSUM") as bc_ps, \
         tc.tile_pool(name="ytps", bufs=1, space="PSUM") as yt_ps_pool:
        for h in range(HPC):
            hp, hl = h // 2, h % 2
            rows = slice(hl * 64, (hl + 1) * 64)
            kT_t, qT_t = qkT16[2 + hp], qkT16[hp]
            expT = []
            for j in range(NT):
                g0 = 128 * j
                nj = T - g0
                e = exp_pool.tile([128, nj], DT, tag=f"exp{j}",
                                  name=f"exp{j}")
                expT.append(e)
                c0 = g0
                while c0 < T:
                    c1 = min(T, (c0 // 512 + 1) * 512)
                    w = c1 - c0
                    ps = sc_ps.tile([128, w], F32, tag="sc", name="sc")
                    nc.tensor.matmul(
                        ps, kT_t[rows, g0:g0 + 128], qT_t[rows, c0:c1],
                        start=True, stop=True)
                    if c0 == g0:
                        nc.vector.tensor_tensor(ps[:, 0:128], ps[:, 0:128],
                                                maskT,
                                                op=mybir.AluOpType.add)
                    nc.scalar.activation(e[:, c0 - g0:c1 - g0], ps,
                                         mybir.ActivationFunctionType.Exp,
                                         scale=0.125)
                    c0 = c1
            # PV: yT accumulation with v_aug stationary
            yt_ps = yt_ps_pool.tile([65, T], F32, tag="ytps", name="ytps")
            for j in range(NT):
                g0 = 128 * j
                c0 = g0
                while c0 < T:
                    c1 = min(T, (c0 // 512 + 1) * 512)
                    bank = c0 // 512
                    j_last = min(4 * bank + 3, NT - 1)
                    nc.tensor.matmul(
                        yt_ps[:, c0:c1], vaug[h][:, j, :],
                        expT[j][:, c0 - g0:c1 - g0],
                        start=(j == 0), stop=(j == j_last))
                    c0 = c1
            # finalize: per-column scale by 1/denominator (row 64)
            densb = fin_pool.tile([1, T], F32, tag="densb", name="densb")
            nc.vector.tensor_copy(densb, yt_ps[64:65, :])
            recr = fin_pool.tile([1, T], F32, tag="recr", name="recr")
            nc.vector.reciprocal(recr, densb)
            for ch in range(NCH):
                sl = slice(ch * 512, (ch + 1) * 512)
                bc = bc_ps.tile([64, 512], F32, tag="bc", name="bc")
                nc.tensor.matmul(bc, ones_row, recr[:, sl],
                                 start=True, stop=True)
                bcs = fin_pool.tile([64, 512], F32, tag="bcs", name="bcs")
                nc.scalar.copy(bcs, bc)
                nc.vector.tensor_tensor(ysb[hp][rows, sl], yt_ps[0:64, sl],
                                        bcs, op=mybir.AluOpType.mult)

    # -------- phase 3: out = ysb.T @ w_o --------
    with tc.tile_pool(name="outsb", bufs=3) as out_sb_pool, \
         tc.tile_pool(name="outps", bufs=4, space="PSUM") as out_ps:
        for r in range(NT):
            for nchk in range(C // 512):
                po = out_ps.tile([128, 512], F32, tag="po", name="po")
                for p in range(2):
                    nc.tensor.matmul(
                        po, ysb[p][:, r * 128:(r + 1) * 128],
                        wo[p][:, nchk * 512:(nchk + 1) * 512],
                        start=(p == 0), stop=(p == 1))
                osb = out_sb_pool.tile([128, 512], F32, tag="osb",
                                       name="osb")
                nc.scalar.copy(osb, po)
                nc.sync.dma_start(
                    out_d[r * 128:(r + 1) * 128,
                          nchk * 512:(nchk + 1) * 512], osb)


def build_nc_v2(T=T_FULL, prec="f32"):
    f16 = prec == "f16"
    DTD = {"f32": F32, "f32r": F32R, "f16": F16}[prec]
    nc = bacc.Bacc("TRN2", target_bir_lowering=False, debug=False,
                   enable_asserts=False, num_devices=N_CORES)
    aps = {}
    # f32r path loads x as f32 (PE transpose + rounding DVE copy produce f32r)
    x_dt = F32 if prec == "f32r" else DTD
    aps["x"] = nc.dram_tensor("x", [T, C], x_dt, kind="ExternalInput").ap()
    aps["w_qkv"] = nc.dram_tensor("w_qkv", [C, QKV_W], DTD,
                                  kind="ExternalInput").ap()
    aps["b_qkvT"] = nc.dram_tensor("b_qkvT", [128, 6], F32,
                                   kind="ExternalInput").ap()
    aps["w_o"] = nc.dram_tensor("w_o", [HPC * D, C], DTD,
                                kind="ExternalInput").ap()
    aps["cos2"] = nc.dram_tensor("cos2", [128, T], F32,
                                 kind="ExternalInput").ap()
    aps["sin2"] = nc.dram_tensor("sin2", [128, T], F32,
                                 kind="ExternalInput").ap()
    aps["pmatT"] = nc.dram_tensor("pmatT", [128, 128], DTD,
                                  kind="ExternalInput").ap()
    aps["out"] = nc.dram_tensor("out", [T, C], F32,
                                kind="ExternalOutput").ap()
    with tile.TileContext(nc) as tc:
        with ExitStack() as ctx:
            emit_attention_v2(ctx, tc, aps, T, prec)
    nc.compile()
    return nc


def shard_inputs_v2(x, w_attn, b_attn, w_proj, T=T_FULL, prec="f32"):
    in_maps = shard_inputs(x, w_attn, b_attn, w_proj, T)
    np_dt = np.float16 if prec == "f16" else np.float32
    for m in in_maps:
        for key in ("x", "w_qkv", "w_o", "pmatT"):
            m[key] = np.ascontiguousarray(m[key].astype(np_dt))
    return in_maps
